# revision 41
# baseline (speedup 1.0000x reference)
"""3-layer GCN + img@pair_embed.T on 8 TRN2 NeuronCores.

Strategy (zero h1 exchange, destination-sharded with redundant halo
recompute, phase-3 overlapped with phase 2; 2.77ms -> 1.25ms vs the old
AllGather-h1 version):
  - Nodes are dealt into 8x28 destination tiles of 128 by descending
    in-degree (round-robin), balancing per-tile edge counts; the host keeps
    the node -> (core, slot) permutation and unpermutes the final output.
  - Every core computes h1 ONLY for the ~13.3k source rows its own layer-2
    edges reference (own slab + halo); layer-1 aggregation A@x for those rows
    runs redundantly per core, with x rows host-pre-gathered into edge-chunk
    order (xg) so layer 1 needs no on-device gathers. This removes the 4x
    29MB h1 AllGathers (1.16ms of collective) of the old version entirely.
  - Aggregation = one-hot matmul over 128-wide dest tiles: S[e, d] = GCN
    norm, aggT[f, d] += G[e, f].T @ S[e, d] per 128-edge chunk.
  - Layer 2 gathers h1 rows (2048 wide, one SWDGE instr per edge chunk) from
    the local h1 buffer, GEMMs with W2 (output kept transposed [feat, dest]),
    then Q = h2 @ W3img (W3img = W3 @ img.T, 64 wide). Q matmuls are emitted
    after the full GEMM loop so PE never waits on Activation.
  - Q is AllGather'd in 4 row-chunks issued as their rows complete. Layer-3
    edges are sub-bucketed by source Q-chunk: gathers/matmuls for sub-bucket
    p run right after AllGather p, interleaved into the phase-2 instruction
    stream; only the last sub-bucket (~25%) remains after phase 2.
  - Everything travels bf16 (PSUM fp32): measured rel err ~4e-3 vs the 2e-2
    gate.
"""

import numpy as np
import ml_dtypes

from concourse import bacc, bass, mybir
from concourse import tile as tile_mod
from concourse.bass_utils import run_bass_kernel_spmd

# Problem shapes (hardcoded per spec nn_GraphModel_26268019982828)
N = 28535
E = 113000
D = 512
H = 2048
B = 64
N_SKIP = 115 + 245

NCORES = 8
P = 128
NT2 = 28               # dest tiles per core
SLAB = NT2 * P         # 3584
NBINS = NCORES * NT2   # 224 global dest tiles
CHUNK_TILES = [4, 4, 4, 4, 4, 3, 3, 2]  # dest tiles per Q AllGather chunk
QCH = len(CHUNK_TILES)
TBE = np.cumsum(CHUNK_TILES)            # chunk end tile (exclusive)
TBS = TBE - np.array(CHUNK_TILES)       # chunk start tile
NFI1 = D // P          # 4
NFI2 = H // P          # 16

f32 = mybir.dt.float32
bf16 = mybir.dt.bfloat16
i32 = mybir.dt.int32
bf = ml_dtypes.bfloat16


def _preprocess(edge_index):
    """Build all per-core tables. Returns dict of host arrays + dims."""
    src0 = np.asarray(edge_index[0], dtype=np.int64)
    dst0 = np.asarray(edge_index[1], dtype=np.int64)
    loops = np.arange(N, dtype=np.int64)
    src = np.concatenate([src0, loops])
    dst = np.concatenate([dst0, loops])
    deg = np.bincount(dst, minlength=N).astype(np.int64)  # >=1 (self loop)
    dinv = 1.0 / np.sqrt(deg.astype(np.float32))
    norm = (dinv[src] * dinv[dst]).astype(np.float32)

    # --- balanced node -> (core, tile, pos) assignment by in-degree ---
    nodes_by_deg = np.argsort(-deg, kind="stable")
    binid = np.arange(N) % NBINS
    binpos = np.arange(N) // NBINS  # < 128
    colmap = np.empty(N, np.int64)
    colmap[nodes_by_deg] = (binid // NT2) * SLAB + (binid % NT2) * P + binpos

    col = colmap[dst]
    dcore = col // SLAB
    dslot = col % SLAB
    dtile = dslot // P
    dpos = dslot % P

    # source position in the chunked-AllGather q layout (uneven chunks)
    scol = colmap[src]
    s_tile = (scol % SLAB) // P
    s_qch = np.searchsorted(TBE, s_tile, side="right")
    ctp = np.array(CHUNK_TILES) * P
    s_qrow = (scol // SLAB) * ctp[s_qch] + (scol % SLAB) - TBS[s_qch] * P

    # --- L2 buckets: sort edges by (dcore, dtile) ---
    bucket = dcore * NT2 + dtile
    eorder = np.argsort(bucket, kind="stable")
    b_s = bucket[eorder]
    src_s = src[eorder]
    norm_s = norm[eorder]
    dpos_s = dpos[eorder]
    counts = np.bincount(b_s, minlength=NBINS)
    ECH2 = int(-(-counts.max() // P))
    starts = np.zeros(NBINS + 1, np.int64)
    np.cumsum(counts, out=starts[1:])
    pos_in = np.arange(len(b_s)) - starts[b_s]
    cidx2 = pos_in // P
    lane2 = pos_in % P
    kcore = b_s // NT2
    ktile = b_s % NT2

    S2 = np.zeros((NCORES, NT2, P, ECH2 * P), np.float32)
    S2[kcore, ktile, lane2, cidx2 * P + dpos_s] = norm_s

    # --- L3 sub-buckets by (dcore, dtile, src q-chunk) ---
    b3 = b_s * QCH + s_qch[eorder]
    e3 = np.argsort(b3, kind="stable")
    b3_s = b3[e3]
    c3counts = np.bincount(b3_s, minlength=NBINS * QCH)
    n3 = -(-c3counts // P)  # chunks per (core,tile,qch)
    n3 = n3.reshape(NCORES, NT2, QCH).max(axis=0)  # uniform across cores
    NCH3 = int(n3.max())
    st3 = np.zeros(NBINS * QCH + 1, np.int64)
    np.cumsum(c3counts, out=st3[1:])
    p3 = np.arange(len(b3_s)) - st3[b3_s]
    cidx3 = p3 // P
    lane3 = p3 % P
    k3 = b3_s // (NT2 * QCH)
    t3 = (b3_s // QCH) % NT2
    q3 = b3_s % QCH
    norm3 = norm_s[e3]
    dpos3 = dpos_s[e3]
    idx3 = np.zeros((NCORES, NT2, QCH, P, NCH3), np.int32)
    S3 = np.zeros((NCORES, NT2, QCH, P, NCH3 * P), np.float32)
    idx3[k3, t3, q3, lane3, cidx3] = s_qrow[eorder][e3].astype(np.int32)
    S3[k3, t3, q3, lane3, cidx3 * P + dpos3] = norm3
    # per-pass merged tables: one idx + one S load per pass instead of 448
    # tiny DMAs (HWDGE fixed cost is 625ns each)
    idx3 = np.ascontiguousarray(
        idx3.transpose(0, 2, 3, 1, 4).reshape(NCORES, QCH, P, NT2 * NCH3)
    )
    S3 = np.ascontiguousarray(
        S3.transpose(0, 2, 3, 1, 4).reshape(NCORES, QCH, P, NT2 * NCH3 * P)
    )

    # --- per-core needed-row sets R_k and L1 tables ---
    dorder = np.argsort(dst, kind="stable")
    src_d = src[dorder]
    norm_d = norm[dorder]
    indptr = np.zeros(N + 1, np.int64)
    np.cumsum(deg, out=indptr[1:])

    per_core = []
    T1_l = []
    for k in range(NCORES):
        m = kcore == k
        Rk = np.unique(src_s[m])
        rk_by_deg = Rk[np.argsort(-deg[Rk], kind="stable")]
        per_core.append((rk_by_deg, m))
        T1_l.append(-(-len(Rk) // P))
    T1 = max(T1_l)

    idx2 = np.zeros((NCORES, NT2, P, ECH2), np.int32)
    ECH1 = 0
    l1_data = []
    for k in range(NCORES):
        rk_by_deg, m = per_core[k]
        nR = len(rk_by_deg)
        i = np.arange(nR)
        t1 = i % T1
        p1 = i // T1
        rpos = np.full(N, -1, np.int64)
        rpos[rk_by_deg] = t1 * P + p1
        idx2[k][ktile[m], lane2[m], cidx2[m]] = rpos[src_s[m]].astype(np.int32)

        lens = deg[rk_by_deg]
        total = int(lens.sum())
        cum = np.zeros(nR + 1, np.int64)
        np.cumsum(lens, out=cum[1:])
        rep_r = np.repeat(i, lens)
        eoff = np.arange(total) - cum[rep_r]
        eidx = indptr[rk_by_deg][rep_r] + eoff
        srcg = src_d[eidx]
        nrm1 = norm_d[eidx]
        et1 = t1[rep_r]
        ep1 = p1[rep_r]
        o1 = np.argsort(et1, kind="stable")
        et1 = et1[o1]; srcg = srcg[o1]; nrm1 = nrm1[o1]; ep1 = ep1[o1]
        c1 = np.bincount(et1, minlength=T1)
        ECH1 = max(ECH1, int(-(-c1.max() // P)))
        l1_data.append((et1, srcg, nrm1, ep1, c1))

    S1_l, l1_gather = [], []
    for k in range(NCORES):
        et1, srcg, nrm1, ep1, c1 = l1_data[k]
        st1 = np.zeros(T1 + 1, np.int64)
        np.cumsum(c1, out=st1[1:])
        pin = np.arange(len(et1)) - st1[et1]
        cidx1 = pin // P
        lane1 = pin % P
        S1 = np.zeros((T1, P, ECH1 * P), np.float32)
        S1[et1, lane1, cidx1 * P + ep1] = nrm1
        S1_l.append(S1)
        l1_gather.append((et1, srcg, cidx1, lane1))

    return dict(
        T1=T1, ECH1=ECH1, ECH2=ECH2, NCH3=NCH3, n3=n3, colmap=colmap,
        idx2=idx2, idx3=idx3, S2=S2, S3=S3, S1_l=S1_l, l1_gather=l1_gather,
    )


def _build(T1, ECH1, ECH2, NCH3, n3, use_b1, use_b2, debug=False):
    nc = bacc.Bacc("TRN2", target_bir_lowering=False, num_devices=NCORES)
    dbg = dict(kind="ExternalOutput") if debug else {}

    xg_t = nc.dram_tensor("xg", [T1, P, ECH1 * D], bf16, kind="ExternalInput")
    s1_t = nc.dram_tensor("S1", [T1, P, ECH1 * P], bf16, kind="ExternalInput")
    s2_t = nc.dram_tensor("S2", [NT2, P, ECH2 * P], bf16, kind="ExternalInput")
    s3_t = nc.dram_tensor("S3", [QCH, P, NT2 * NCH3 * P], bf16, kind="ExternalInput")
    idx2_t = nc.dram_tensor("idx2", [NT2, P, ECH2], i32, kind="ExternalInput")
    idx3_t = nc.dram_tensor("idx3", [QCH, P, NT2 * NCH3], i32, kind="ExternalInput")
    w1_t = nc.dram_tensor("W1", [D, H], bf16, kind="ExternalInput")
    w2_t = nc.dram_tensor("W2", [H, H], bf16, kind="ExternalInput")
    w3i_t = nc.dram_tensor("W3img", [H, B], bf16, kind="ExternalInput")
    if use_b1:
        b1_t = nc.dram_tensor("b1", [1, H], bf16, kind="ExternalInput")
    if use_b2:
        b2_t = nc.dram_tensor("b2", [P, NFI2], f32, kind="ExternalInput")

    h1loc = nc.dram_tensor("h1loc", [T1 * P, H], bf16, **dbg)
    q_slab = nc.dram_tensor("q_slab", [SLAB, B], bf16)
    if debug:
        q_dbg = nc.dram_tensor("q_dbg", [SLAB, B], bf16, kind="ExternalOutput")
    qf = [
        nc.dram_tensor(
            f"qf{p}", [NCORES * CHUNK_TILES[p] * P, B], bf16,
            addr_space="Shared",
        )
        for p in range(QCH)
    ]
    out_t = nc.dram_tensor("out", [B, SLAB], f32, kind="ExternalOutput")

    rg = [list(range(NCORES))]
    relu = mybir.ActivationFunctionType.Relu

    from contextlib import ExitStack

    with tile_mod.TileContext(nc) as tc, ExitStack() as st:
        if True:
            wp = st.enter_context(tc.tile_pool(name="w", bufs=20))
            w3p = st.enter_context(tc.tile_pool(name="w3", bufs=16))
            xp = st.enter_context(tc.tile_pool(name="xg", bufs=3))
            s1p = st.enter_context(tc.tile_pool(name="s1", bufs=3))
            s2p = st.enter_context(tc.tile_pool(name="s2", bufs=3))
            s3p = st.enter_context(tc.tile_pool(name="s3", bufs=2))
            hp = st.enter_context(tc.tile_pool(name="h1t", bufs=2))
            ap = st.enter_context(tc.tile_pool(name="agg", bufs=2))
            gp = st.enter_context(tc.tile_pool(name="g", bufs=8))
            g3p = st.enter_context(tc.tile_pool(name="g3", bufs=3))
            h2p = st.enter_context(tc.tile_pool(name="h2c", bufs=20))
            mp = st.enter_context(tc.tile_pool(name="small", bufs=4))
            accp = st.enter_context(tc.tile_pool(name="acc", bufs=1))
            cp = st.enter_context(tc.tile_pool(name="consts", bufs=1))
            # 8 PSUM banks total: psA 2 (pz1/pz2), psB 2 (pa1/pq/pp3), pa2 4
            psA = st.enter_context(tc.tile_pool(name="psA", bufs=2, space="PSUM"))
            psB = st.enter_context(tc.tile_pool(name="psB", bufs=2, space="PSUM"))
            pa2p = st.enter_context(tc.tile_pool(name="pa2", bufs=1, space="PSUM"))
            pa1p = psB
            pz1p = psA
            pz2p = psA
            pqdp = psB
            # tile-0 inputs first: the first aggregation only needs xg/S1,
            # so don't queue it behind the W1 loads
            xg0 = xp.tile([P, ECH1 * D], bf16, tag="xg", name="xg0")
            nc.sync.dma_start(out=xg0[:], in_=xg_t[0])
            s10 = s1p.tile([P, ECH1 * P], bf16, tag="s1", name="s10")
            nc.sync.dma_start(out=s10[:], in_=s1_t[0])
            # resident weights
            w1sb = []
            for fi in range(NFI1):
                w = wp.tile([P, H], bf16, tag="w", name="w1sb")
                nc.sync.dma_start(out=w[:], in_=w1_t[fi * P : (fi + 1) * P, :])
                w1sb.append(w)
            w2sb = [wp.tile([P, H], bf16, tag="w", name="w2sb") for _ in range(NFI2)]
            w3sb = [w3p.tile([P, B], bf16, tag="w3", name="w3sb") for _ in range(NFI2)]

            def load_w23(i):
                # deferred + spread: phase 1 only needs W1; a block of W2/W3
                # loads anywhere stalls the in-order xg stream ~23us, so emit
                # one load per phase-1 tile
                if 0 <= i < NFI2:
                    nc.sync.dma_start(
                        out=w2sb[i][:], in_=w2_t[i * P : (i + 1) * P, :]
                    )
                elif NFI2 <= i < 2 * NFI2:
                    fo = i - NFI2
                    nc.sync.dma_start(
                        out=w3sb[fo][:], in_=w3i_t[fo * P : (fo + 1) * P, :]
                    )
            if use_b1:
                b1sb = cp.tile([1, H], bf16)
                nc.sync.dma_start(out=b1sb[:], in_=b1_t[:])
                ones1 = cp.tile([1, P], bf16)
                nc.gpsimd.memset(ones1[:], 1.0)
            if use_b2:
                b2sb = cp.tile([P, NFI2], f32)
                nc.sync.dma_start(out=b2sb[:], in_=b2_t[:])

            # phase-3 SBUF accumulator [B, SLAB] f32 and per-tile state
            acc = accp.tile([B, SLAB], f32)
            acc_started = [False] * NT2

            # ---------------- Phase 1: h1 for all needed rows ----------------
            for t in range(T1):
                if t == 0:
                    xg_s, s_s = xg0, s10
                else:
                    xg_s = xp.tile([P, ECH1 * D], bf16, tag="xg")
                    nc.sync.dma_start(out=xg_s[:], in_=xg_t[t])
                    s_s = s1p.tile([P, ECH1 * P], bf16, tag="s1")
                    nc.sync.dma_start(out=s_s[:], in_=s1_t[t])

                pa = pa1p.tile([P, D], f32, tag="b", name="pa1")
                for fi in range(NFI1):
                    for c in range(ECH1):
                        # sequential accumulation chain per PSUM slice:
                        # interleaved chains miscompile on walrus
                        nc.tensor.matmul(
                            out=pa[:, fi * P : (fi + 1) * P],
                            lhsT=xg_s[:, c * D + fi * P : c * D + (fi + 1) * P],
                            rhs=s_s[:, c * P : (c + 1) * P],
                            start=(c == 0),
                            stop=(c == ECH1 - 1),
                        )
                aggT = ap.tile([P, D], bf16, tag="agg", name="aggT1")
                nc.vector.tensor_copy(out=aggT[:], in_=pa[:])

                h1t = hp.tile([P, H], bf16, tag="h1t")
                for fo in range(NFI1):
                    pz = pz1p.tile([P, D], f32, tag="z", name="pz1")
                    if use_b1:
                        nc.tensor.matmul(
                            out=pz[:], lhsT=ones1[:1, :],
                            rhs=b1sb[:1, fo * D : (fo + 1) * D],
                            start=True, stop=False,
                        )
                    for fi in range(NFI1):
                        nc.tensor.matmul(
                            out=pz[:],
                            lhsT=aggT[:, fi * P : (fi + 1) * P],
                            rhs=w1sb[fi][:, fo * D : (fo + 1) * D],
                            start=(fi == 0 and not use_b1),
                            stop=(fi == NFI1 - 1),
                        )
                    nc.scalar.activation(
                        out=h1t[:, fo * D : (fo + 1) * D], in_=pz[:], func=relu
                    )
                nc.sync.dma_start(out=h1loc[t * P : (t + 1) * P, :], in_=h1t[:])
                if t % 3 == 1:
                    load_w23(t // 3)

            # ---------------- Phase 3 helper (emitted interleaved) ----------
            p3_tiles = {}  # pass -> (idx tile, s3 tile)

            def phase3_load(p):
                idx_s = mp.tile([P, NT2 * NCH3], i32, tag="idx3")
                nc.sync.dma_start(out=idx_s[:], in_=idx3_t[p])
                s_s = s3p.tile([P, NT2 * NCH3 * P], bf16, tag="s3")
                nc.sync.dma_start(out=s_s[:], in_=s3_t[p])
                p3_tiles[p] = (idx_s, s_s)

            def phase3_pass(t, p):
                if n3[t][p] == 0:
                    return
                if p not in p3_tiles:
                    phase3_load(p)
                idx_s, s_s = p3_tiles[p]
                pp3 = pqdp.tile([B, P], f32, tag="b", name="pp3")
                for c in range(int(n3[t][p])):
                    col = t * NCH3 + c
                    g = g3p.tile([P, B], bf16, tag="g3")
                    nc.gpsimd.indirect_dma_start(
                        out=g[:],
                        out_offset=None,
                        in_=qf[p][:],
                        in_offset=bass.IndirectOffsetOnAxis(
                            ap=idx_s[:, col : col + 1], axis=0
                        ),
                    )
                    nc.tensor.matmul(
                        out=pp3[:],
                        lhsT=g[:],
                        rhs=s_s[:, col * P : (col + 1) * P],
                        start=(c == 0),
                        stop=(c == int(n3[t][p]) - 1),
                    )
                dstv = acc[:, t * P : (t + 1) * P]
                if not acc_started[t]:
                    nc.vector.tensor_copy(out=dstv, in_=pp3[:])
                    acc_started[t] = True
                else:
                    nc.vector.tensor_tensor(
                        out=dstv, in0=dstv, in1=pp3[:],
                        op=mybir.AluOpType.add,
                    )

            # ---------------- Phase 2: layer 2 + Q (+ interleaved phase 3) --
            # Software-pipelined: iteration t aggregates tile t and GEMMs
            # tile t-1, so the PSUM->SBUF copy hides under the previous GEMM.
            p3_queue = []  # (ready_iter, t3, p)
            aggT2s = [None, None]
            for t in range(NT2 + 1):
                if t < NT2:
                    idx_s = mp.tile([P, ECH2], i32, tag="idx")
                    nc.sync.dma_start(out=idx_s[:], in_=idx2_t[t])
                    s_s = s2p.tile([P, ECH2 * P], bf16, tag="s2")
                    nc.sync.dma_start(out=s_s[:], in_=s2_t[t])

                    pa2 = pa2p.tile([P, H], f32, tag="pa2", name="pa2")
                    gs = []
                    for c in range(ECH2):
                        g = gp.tile([P, H], bf16, tag="g")
                        nc.gpsimd.indirect_dma_start(
                            out=g[:],
                            out_offset=None,
                            in_=h1loc[:],
                            in_offset=bass.IndirectOffsetOnAxis(
                                ap=idx_s[:, c : c + 1], axis=0
                            ),
                        )
                        gs.append(g)
                    for j in range(NFI2):
                        for c in range(ECH2):
                            nc.tensor.matmul(
                                out=pa2[:, j * P : (j + 1) * P],
                                lhsT=gs[c][:, j * P : (j + 1) * P],
                                rhs=s_s[:, c * P : (c + 1) * P],
                                start=(c == 0),
                                stop=(c == ECH2 - 1),
                            )
                    aggT2 = ap.tile([P, H], bf16, tag="agg", name="aggT2")
                    nc.vector.tensor_copy(out=aggT2[:], in_=pa2[:])
                    aggT2s[t % 2] = aggT2

                if t >= 1:
                    tg = t - 1
                    aggT2g = aggT2s[tg % 2]
                    h2cs = []
                    for fo in range(NFI2):
                        pz = pz2p.tile([P, P], f32, tag="z", name="pz2")
                        for fi in range(NFI2):
                            nc.tensor.matmul(
                                out=pz[:],
                                lhsT=w2sb[fi][:, fo * P : (fo + 1) * P],
                                rhs=aggT2g[:, fi * P : (fi + 1) * P],
                                start=(fi == 0),
                                stop=(fi == NFI2 - 1),
                            )
                        h2c = h2p.tile([P, P], bf16, tag="h2c")
                        if use_b2:
                            nc.scalar.activation(
                                out=h2c[:], in_=pz[:], func=relu,
                                bias=b2sb[:, fo : fo + 1],
                            )
                        else:
                            nc.scalar.activation(out=h2c[:], in_=pz[:], func=relu)
                        h2cs.append(h2c)
                    pq = pqdp.tile([P, B], f32, tag="b", name="pq")
                    for fo in range(NFI2):
                        nc.tensor.matmul(
                            out=pq[:],
                            lhsT=h2cs[fo][:],
                            rhs=w3sb[fo][:],
                            start=(fo == 0),
                            stop=(fo == NFI2 - 1),
                        )
                    qn = mp.tile([P, B], bf16, tag="qn")
                    nc.vector.tensor_copy(out=qn[:], in_=pq[:])
                    nc.sync.dma_start(
                        out=q_slab[tg * P : (tg + 1) * P, :], in_=qn[:]
                    )
                    if debug:
                        nc.sync.dma_start(
                            out=q_dbg[tg * P : (tg + 1) * P, :], in_=qn[:]
                        )

                    if (tg + 1) in TBE:
                        ch = int(np.searchsorted(TBE, tg + 1))
                        nc.gpsimd.collective_compute(
                            "AllGather",
                            mybir.AluOpType.bypass,
                            replica_groups=rg,
                            ins=[q_slab[TBS[ch] * P : TBE[ch] * P, :]],
                            outs=[qf[ch][:]],
                        )
                        phase3_load(ch)
                        if ch < QCH - 1:
                            # drain only 2+ iterations later so the AllGather
                            # is done before Pool reaches these gathers
                            # (in-order SEQ would head-of-line block phase 2)
                            p3_queue.extend(
                                (t + 2, t3, ch) for t3 in range(NT2)
                            )

                drained = 0
                dmax = 11 if t >= 16 else 7
                while p3_queue and p3_queue[0][0] <= t and drained < dmax:
                    _, t3, pch = p3_queue.pop(0)
                    phase3_pass(t3, pch)
                    drained += 1

            for _, t3, pch in p3_queue:
                phase3_pass(t3, pch)
            for t3 in range(NT2):
                phase3_pass(t3, QCH - 1)

            nc.sync.dma_start(out=out_t[:], in_=acc[:])

    nc.finalize()
    return nc


_CACHE: dict = {}


def kernel(**inputs: np.ndarray) -> np.ndarray:
    nodes = np.asarray(inputs["nodes"], dtype=np.float32)
    edge_index = np.asarray(inputs["edge_index"])
    img = np.asarray(inputs["img"], dtype=np.float32)
    W1 = np.asarray(inputs["W1"], dtype=np.float32)
    b1 = np.asarray(inputs["b1"], dtype=np.float32)
    W2 = np.asarray(inputs["W2"], dtype=np.float32)
    b2 = np.asarray(inputs["b2"], dtype=np.float32)
    W3 = np.asarray(inputs["W3"], dtype=np.float32)
    b3 = np.asarray(inputs["b3"], dtype=np.float32)

    pp = _preprocess(edge_index)
    T1, ECH1, ECH2, NCH3 = pp["T1"], pp["ECH1"], pp["ECH2"], pp["NCH3"]
    use_b1 = bool(np.any(b1))
    use_b2 = bool(np.any(b2))

    key = (T1, ECH1, ECH2, NCH3, pp["n3"].tobytes(), use_b1, use_b2)
    if key not in _CACHE:
        _CACHE[key] = _build(T1, ECH1, ECH2, NCH3, pp["n3"], use_b1, use_b2)
    nc = _CACHE[key]

    nodes_bf = nodes.astype(bf)
    w1_bf = W1.astype(bf)
    w2_bf = W2.astype(bf)
    w3img = (W3 @ img.T).astype(bf)  # [H, B]
    outbias = img @ b3  # [B]

    in_maps = []
    for k in range(NCORES):
        et1, srcg, cidx1, lane1 = pp["l1_gather"][k]
        xg = np.zeros((T1, P, ECH1, D), bf)
        xg[et1, lane1, cidx1] = nodes_bf[srcg]
        m = {
            "xg": xg.reshape(T1, P, ECH1 * D),
            "S1": pp["S1_l"][k].astype(bf),
            "S2": np.ascontiguousarray(pp["S2"][k]).astype(bf),
            "S3": np.ascontiguousarray(pp["S3"][k]).astype(bf),
            "idx2": np.ascontiguousarray(pp["idx2"][k]),
            "idx3": np.ascontiguousarray(pp["idx3"][k]),
            "W1": w1_bf,
            "W2": w2_bf,
            "W3img": w3img,
        }
        if use_b1:
            m["b1"] = b1.reshape(1, H).astype(bf)
        if use_b2:
            m["b2"] = np.ascontiguousarray(b2.reshape(NFI2, P).T).astype(np.float32)
        in_maps.append(m)

    res = run_bass_kernel_spmd(nc, in_maps, core_ids=list(range(NCORES)))

    full = np.concatenate([res.results[k]["out"] for k in range(NCORES)], axis=1)
    cols = pp["colmap"][np.arange(N_SKIP, N)]
    out = full[:, cols] + outbias[:, None]
    return out.astype(np.float32)


if __name__ == "__main__":
    rng = np.random.default_rng(0)
    ins = {
        "nodes": rng.standard_normal((N, D)).astype(np.float32),
        "edge_index": rng.integers(0, N, size=(2, E)).astype(np.int64),
        "img": rng.standard_normal((B, D)).astype(np.float32),
        "W1": (rng.standard_normal((D, H)) * 0.02).astype(np.float32),
        "b1": np.zeros(H, np.float32),
        "W2": (rng.standard_normal((H, H)) * 0.02).astype(np.float32),
        "b2": np.zeros(H, np.float32),
        "W3": (rng.standard_normal((H, D)) * 0.02).astype(np.float32),
        "b3": np.zeros(D, np.float32),
    }
    out = kernel(**ins)
    print("out", out.shape, out.dtype, np.abs(out).mean())


# revision 42
# speedup vs baseline: 1.0013x; 1.0013x over previous
"""3-layer GCN + img@pair_embed.T on 8 TRN2 NeuronCores.

Strategy (zero h1 exchange, destination-sharded with redundant halo
recompute, phase-3 overlapped with phase 2; 2.77ms -> 1.25ms vs the old
AllGather-h1 version):
  - Nodes are dealt into 8x28 destination tiles of 128 by descending
    in-degree (round-robin), balancing per-tile edge counts; the host keeps
    the node -> (core, slot) permutation and unpermutes the final output.
  - Every core computes h1 ONLY for the ~13.3k source rows its own layer-2
    edges reference (own slab + halo); layer-1 aggregation A@x for those rows
    runs redundantly per core, with x rows host-pre-gathered into edge-chunk
    order (xg) so layer 1 needs no on-device gathers. This removes the 4x
    29MB h1 AllGathers (1.16ms of collective) of the old version entirely.
  - Aggregation = one-hot matmul over 128-wide dest tiles: S[e, d] = GCN
    norm, aggT[f, d] += G[e, f].T @ S[e, d] per 128-edge chunk.
  - Layer 2 gathers h1 rows (2048 wide, one SWDGE instr per edge chunk) from
    the local h1 buffer, GEMMs with W2 (output kept transposed [feat, dest]),
    then Q = h2 @ W3img (W3img = W3 @ img.T, 64 wide). Q matmuls are emitted
    after the full GEMM loop so PE never waits on Activation.
  - Q is AllGather'd in 4 row-chunks issued as their rows complete. Layer-3
    edges are sub-bucketed by source Q-chunk: gathers/matmuls for sub-bucket
    p run right after AllGather p, interleaved into the phase-2 instruction
    stream; only the last sub-bucket (~25%) remains after phase 2.
  - Everything travels bf16 (PSUM fp32): measured rel err ~4e-3 vs the 2e-2
    gate.
"""

import numpy as np
import ml_dtypes

from concourse import bacc, bass, mybir
from concourse import tile as tile_mod
from concourse.bass_utils import run_bass_kernel_spmd

# Problem shapes (hardcoded per spec nn_GraphModel_26268019982828)
N = 28535
E = 113000
D = 512
H = 2048
B = 64
N_SKIP = 115 + 245

NCORES = 8
P = 128
NT2 = 28               # dest tiles per core
SLAB = NT2 * P         # 3584
NBINS = NCORES * NT2   # 224 global dest tiles
CHUNK_TILES = [4, 4, 4, 4, 4, 3, 3, 2]  # dest tiles per Q AllGather chunk
QCH = len(CHUNK_TILES)
TBE = np.cumsum(CHUNK_TILES)            # chunk end tile (exclusive)
TBS = TBE - np.array(CHUNK_TILES)       # chunk start tile
NFI1 = D // P          # 4
NFI2 = H // P          # 16

f32 = mybir.dt.float32
bf16 = mybir.dt.bfloat16
i32 = mybir.dt.int32
bf = ml_dtypes.bfloat16


def _preprocess(edge_index):
    """Build all per-core tables. Returns dict of host arrays + dims."""
    src0 = np.asarray(edge_index[0], dtype=np.int64)
    dst0 = np.asarray(edge_index[1], dtype=np.int64)
    loops = np.arange(N, dtype=np.int64)
    src = np.concatenate([src0, loops])
    dst = np.concatenate([dst0, loops])
    deg = np.bincount(dst, minlength=N).astype(np.int64)  # >=1 (self loop)
    dinv = 1.0 / np.sqrt(deg.astype(np.float32))
    norm = (dinv[src] * dinv[dst]).astype(np.float32)

    # --- balanced node -> (core, tile, pos) assignment by in-degree ---
    nodes_by_deg = np.argsort(-deg, kind="stable")
    binid = np.arange(N) % NBINS
    binpos = np.arange(N) // NBINS  # < 128
    colmap = np.empty(N, np.int64)
    colmap[nodes_by_deg] = (binid // NT2) * SLAB + (binid % NT2) * P + binpos

    col = colmap[dst]
    dcore = col // SLAB
    dslot = col % SLAB
    dtile = dslot // P
    dpos = dslot % P

    # source position in the chunked-AllGather q layout (uneven chunks)
    scol = colmap[src]
    s_tile = (scol % SLAB) // P
    s_qch = np.searchsorted(TBE, s_tile, side="right")
    ctp = np.array(CHUNK_TILES) * P
    s_qrow = (scol // SLAB) * ctp[s_qch] + (scol % SLAB) - TBS[s_qch] * P

    # --- L2 buckets: sort edges by (dcore, dtile) ---
    bucket = dcore * NT2 + dtile
    eorder = np.argsort(bucket, kind="stable")
    b_s = bucket[eorder]
    src_s = src[eorder]
    norm_s = norm[eorder]
    dpos_s = dpos[eorder]
    counts = np.bincount(b_s, minlength=NBINS)
    ECH2 = int(-(-counts.max() // P))
    starts = np.zeros(NBINS + 1, np.int64)
    np.cumsum(counts, out=starts[1:])
    pos_in = np.arange(len(b_s)) - starts[b_s]
    cidx2 = pos_in // P
    lane2 = pos_in % P
    kcore = b_s // NT2
    ktile = b_s % NT2

    S2 = np.zeros((NCORES, NT2, P, ECH2 * P), np.float32)
    S2[kcore, ktile, lane2, cidx2 * P + dpos_s] = norm_s

    # --- L3 sub-buckets by (dcore, dtile, src q-chunk) ---
    b3 = b_s * QCH + s_qch[eorder]
    e3 = np.argsort(b3, kind="stable")
    b3_s = b3[e3]
    c3counts = np.bincount(b3_s, minlength=NBINS * QCH)
    n3 = -(-c3counts // P)  # chunks per (core,tile,qch)
    n3 = n3.reshape(NCORES, NT2, QCH).max(axis=0)  # uniform across cores
    NCH3 = int(n3.max())
    st3 = np.zeros(NBINS * QCH + 1, np.int64)
    np.cumsum(c3counts, out=st3[1:])
    p3 = np.arange(len(b3_s)) - st3[b3_s]
    cidx3 = p3 // P
    lane3 = p3 % P
    k3 = b3_s // (NT2 * QCH)
    t3 = (b3_s // QCH) % NT2
    q3 = b3_s % QCH
    norm3 = norm_s[e3]
    dpos3 = dpos_s[e3]
    idx3 = np.zeros((NCORES, NT2, QCH, P, NCH3), np.int32)
    S3 = np.zeros((NCORES, NT2, QCH, P, NCH3 * P), np.float32)
    idx3[k3, t3, q3, lane3, cidx3] = s_qrow[eorder][e3].astype(np.int32)
    S3[k3, t3, q3, lane3, cidx3 * P + dpos3] = norm3
    # per-pass merged tables: one idx + one S load per pass instead of 448
    # tiny DMAs (HWDGE fixed cost is 625ns each)
    idx3 = np.ascontiguousarray(
        idx3.transpose(0, 2, 3, 1, 4).reshape(NCORES, QCH, P, NT2 * NCH3)
    )
    S3 = np.ascontiguousarray(
        S3.transpose(0, 2, 3, 1, 4).reshape(NCORES, QCH, P, NT2 * NCH3 * P)
    )

    # --- per-core needed-row sets R_k and L1 tables ---
    dorder = np.argsort(dst, kind="stable")
    src_d = src[dorder]
    norm_d = norm[dorder]
    indptr = np.zeros(N + 1, np.int64)
    np.cumsum(deg, out=indptr[1:])

    per_core = []
    T1_l = []
    for k in range(NCORES):
        m = kcore == k
        Rk = np.unique(src_s[m])
        rk_by_deg = Rk[np.argsort(-deg[Rk], kind="stable")]
        per_core.append((rk_by_deg, m))
        T1_l.append(-(-len(Rk) // P))
    T1 = max(T1_l)

    idx2 = np.zeros((NCORES, NT2, P, ECH2), np.int32)
    ECH1 = 0
    l1_data = []
    for k in range(NCORES):
        rk_by_deg, m = per_core[k]
        nR = len(rk_by_deg)
        i = np.arange(nR)
        t1 = i % T1
        p1 = i // T1
        rpos = np.full(N, -1, np.int64)
        rpos[rk_by_deg] = t1 * P + p1
        idx2[k][ktile[m], lane2[m], cidx2[m]] = rpos[src_s[m]].astype(np.int32)

        lens = deg[rk_by_deg]
        total = int(lens.sum())
        cum = np.zeros(nR + 1, np.int64)
        np.cumsum(lens, out=cum[1:])
        rep_r = np.repeat(i, lens)
        eoff = np.arange(total) - cum[rep_r]
        eidx = indptr[rk_by_deg][rep_r] + eoff
        srcg = src_d[eidx]
        nrm1 = norm_d[eidx]
        et1 = t1[rep_r]
        ep1 = p1[rep_r]
        o1 = np.argsort(et1, kind="stable")
        et1 = et1[o1]; srcg = srcg[o1]; nrm1 = nrm1[o1]; ep1 = ep1[o1]
        c1 = np.bincount(et1, minlength=T1)
        ECH1 = max(ECH1, int(-(-c1.max() // P)))
        l1_data.append((et1, srcg, nrm1, ep1, c1))

    S1_l, l1_gather = [], []
    for k in range(NCORES):
        et1, srcg, nrm1, ep1, c1 = l1_data[k]
        st1 = np.zeros(T1 + 1, np.int64)
        np.cumsum(c1, out=st1[1:])
        pin = np.arange(len(et1)) - st1[et1]
        cidx1 = pin // P
        lane1 = pin % P
        S1 = np.zeros((T1, P, ECH1 * P), np.float32)
        S1[et1, lane1, cidx1 * P + ep1] = nrm1
        S1_l.append(S1)
        l1_gather.append((et1, srcg, cidx1, lane1))

    return dict(
        T1=T1, ECH1=ECH1, ECH2=ECH2, NCH3=NCH3, n3=n3, colmap=colmap,
        idx2=idx2, idx3=idx3, S2=S2, S3=S3, S1_l=S1_l, l1_gather=l1_gather,
    )


def _build(T1, ECH1, ECH2, NCH3, n3, use_b1, use_b2, debug=False):
    nc = bacc.Bacc("TRN2", target_bir_lowering=False, num_devices=NCORES)
    dbg = dict(kind="ExternalOutput") if debug else {}

    xg_t = nc.dram_tensor("xg", [T1, P, ECH1 * D], bf16, kind="ExternalInput")
    s1_t = nc.dram_tensor("S1", [T1, P, ECH1 * P], bf16, kind="ExternalInput")
    s2_t = nc.dram_tensor("S2", [NT2, P, ECH2 * P], bf16, kind="ExternalInput")
    s3_t = nc.dram_tensor("S3", [QCH, P, NT2 * NCH3 * P], bf16, kind="ExternalInput")
    idx2_t = nc.dram_tensor("idx2", [NT2, P, ECH2], i32, kind="ExternalInput")
    idx3_t = nc.dram_tensor("idx3", [QCH, P, NT2 * NCH3], i32, kind="ExternalInput")
    w1_t = nc.dram_tensor("W1", [D, H], bf16, kind="ExternalInput")
    w2_t = nc.dram_tensor("W2", [H, H], bf16, kind="ExternalInput")
    w3i_t = nc.dram_tensor("W3img", [H, B], bf16, kind="ExternalInput")
    if use_b1:
        b1_t = nc.dram_tensor("b1", [1, H], bf16, kind="ExternalInput")
    if use_b2:
        b2_t = nc.dram_tensor("b2", [P, NFI2], f32, kind="ExternalInput")

    h1loc = nc.dram_tensor("h1loc", [T1 * P, H], bf16, **dbg)
    q_slab = nc.dram_tensor("q_slab", [SLAB, B], bf16)
    if debug:
        q_dbg = nc.dram_tensor("q_dbg", [SLAB, B], bf16, kind="ExternalOutput")
    qf = [
        nc.dram_tensor(
            f"qf{p}", [NCORES * CHUNK_TILES[p] * P, B], bf16,
            addr_space="Shared",
        )
        for p in range(QCH)
    ]
    out_t = nc.dram_tensor("out", [B, SLAB], f32, kind="ExternalOutput")

    rg = [list(range(NCORES))]
    relu = mybir.ActivationFunctionType.Relu

    from contextlib import ExitStack

    with tile_mod.TileContext(nc) as tc, ExitStack() as st:
        if True:
            wp = st.enter_context(tc.tile_pool(name="w", bufs=20))
            w3p = st.enter_context(tc.tile_pool(name="w3", bufs=16))
            xp = st.enter_context(tc.tile_pool(name="xg", bufs=3))
            s1p = st.enter_context(tc.tile_pool(name="s1", bufs=3))
            s2p = st.enter_context(tc.tile_pool(name="s2", bufs=3))
            s3p = st.enter_context(tc.tile_pool(name="s3", bufs=2))
            hp = st.enter_context(tc.tile_pool(name="h1t", bufs=2))
            ap = st.enter_context(tc.tile_pool(name="agg", bufs=2))
            gp = st.enter_context(tc.tile_pool(name="g", bufs=8))
            g3p = st.enter_context(tc.tile_pool(name="g3", bufs=3))
            h2p = st.enter_context(tc.tile_pool(name="h2c", bufs=24))
            mp = st.enter_context(tc.tile_pool(name="small", bufs=4))
            accp = st.enter_context(tc.tile_pool(name="acc", bufs=1))
            cp = st.enter_context(tc.tile_pool(name="consts", bufs=1))
            # 8 PSUM banks total: psA 2 (pz1/pz2), psB 2 (pa1/pq/pp3), pa2 4
            psA = st.enter_context(tc.tile_pool(name="psA", bufs=2, space="PSUM"))
            psB = st.enter_context(tc.tile_pool(name="psB", bufs=2, space="PSUM"))
            pa2p = st.enter_context(tc.tile_pool(name="pa2", bufs=1, space="PSUM"))
            pa1p = psB
            pz1p = psA
            pz2p = psA
            pqdp = psB
            # tile-0 inputs first: the first aggregation only needs xg/S1,
            # so don't queue it behind the W1 loads
            xg0 = xp.tile([P, ECH1 * D], bf16, tag="xg", name="xg0")
            nc.sync.dma_start(out=xg0[:], in_=xg_t[0])
            s10 = s1p.tile([P, ECH1 * P], bf16, tag="s1", name="s10")
            nc.sync.dma_start(out=s10[:], in_=s1_t[0])
            # resident weights
            w1sb = []
            for fi in range(NFI1):
                w = wp.tile([P, H], bf16, tag="w", name="w1sb")
                nc.sync.dma_start(out=w[:], in_=w1_t[fi * P : (fi + 1) * P, :])
                w1sb.append(w)
            w2sb = [wp.tile([P, H], bf16, tag="w", name="w2sb") for _ in range(NFI2)]
            w3sb = [w3p.tile([P, B], bf16, tag="w3", name="w3sb") for _ in range(NFI2)]

            def load_w23(i):
                # deferred + spread: phase 1 only needs W1; a block of W2/W3
                # loads anywhere stalls the in-order xg stream ~23us, so emit
                # one load per phase-1 tile
                if 0 <= i < NFI2:
                    nc.sync.dma_start(
                        out=w2sb[i][:], in_=w2_t[i * P : (i + 1) * P, :]
                    )
                elif NFI2 <= i < 2 * NFI2:
                    fo = i - NFI2
                    nc.sync.dma_start(
                        out=w3sb[fo][:], in_=w3i_t[fo * P : (fo + 1) * P, :]
                    )
            if use_b1:
                b1sb = cp.tile([1, H], bf16)
                nc.sync.dma_start(out=b1sb[:], in_=b1_t[:])
                ones1 = cp.tile([1, P], bf16)
                nc.gpsimd.memset(ones1[:], 1.0)
            if use_b2:
                b2sb = cp.tile([P, NFI2], f32)
                nc.sync.dma_start(out=b2sb[:], in_=b2_t[:])

            # phase-3 SBUF accumulator [B, SLAB] f32 and per-tile state
            acc = accp.tile([B, SLAB], f32)
            acc_started = [False] * NT2

            # ---------------- Phase 1: h1 for all needed rows ----------------
            for t in range(T1):
                if t == 0:
                    xg_s, s_s = xg0, s10
                else:
                    xg_s = xp.tile([P, ECH1 * D], bf16, tag="xg")
                    nc.sync.dma_start(out=xg_s[:], in_=xg_t[t])
                    s_s = s1p.tile([P, ECH1 * P], bf16, tag="s1")
                    nc.sync.dma_start(out=s_s[:], in_=s1_t[t])

                pa = pa1p.tile([P, D], f32, tag="b", name="pa1")
                for fi in range(NFI1):
                    for c in range(ECH1):
                        # sequential accumulation chain per PSUM slice:
                        # interleaved chains miscompile on walrus
                        nc.tensor.matmul(
                            out=pa[:, fi * P : (fi + 1) * P],
                            lhsT=xg_s[:, c * D + fi * P : c * D + (fi + 1) * P],
                            rhs=s_s[:, c * P : (c + 1) * P],
                            start=(c == 0),
                            stop=(c == ECH1 - 1),
                        )
                aggT = ap.tile([P, D], bf16, tag="agg", name="aggT1")
                nc.vector.tensor_copy(out=aggT[:], in_=pa[:])

                h1t = hp.tile([P, H], bf16, tag="h1t")
                for fo in range(NFI1):
                    pz = pz1p.tile([P, D], f32, tag="z", name="pz1")
                    if use_b1:
                        nc.tensor.matmul(
                            out=pz[:], lhsT=ones1[:1, :],
                            rhs=b1sb[:1, fo * D : (fo + 1) * D],
                            start=True, stop=False,
                        )
                    for fi in range(NFI1):
                        nc.tensor.matmul(
                            out=pz[:],
                            lhsT=aggT[:, fi * P : (fi + 1) * P],
                            rhs=w1sb[fi][:, fo * D : (fo + 1) * D],
                            start=(fi == 0 and not use_b1),
                            stop=(fi == NFI1 - 1),
                        )
                    nc.scalar.activation(
                        out=h1t[:, fo * D : (fo + 1) * D], in_=pz[:], func=relu
                    )
                nc.sync.dma_start(out=h1loc[t * P : (t + 1) * P, :], in_=h1t[:])
                if t % 3 == 1:
                    load_w23(t // 3)

            # ---------------- Phase 3 helper (emitted interleaved) ----------
            p3_tiles = {}  # pass -> (idx tile, s3 tile)

            def phase3_load(p):
                idx_s = mp.tile([P, NT2 * NCH3], i32, tag="idx3")
                nc.sync.dma_start(out=idx_s[:], in_=idx3_t[p])
                s_s = s3p.tile([P, NT2 * NCH3 * P], bf16, tag="s3")
                nc.sync.dma_start(out=s_s[:], in_=s3_t[p])
                p3_tiles[p] = (idx_s, s_s)

            def phase3_pass(t, p):
                if n3[t][p] == 0:
                    return
                if p not in p3_tiles:
                    phase3_load(p)
                idx_s, s_s = p3_tiles[p]
                pp3 = pqdp.tile([B, P], f32, tag="b", name="pp3")
                for c in range(int(n3[t][p])):
                    col = t * NCH3 + c
                    g = g3p.tile([P, B], bf16, tag="g3")
                    nc.gpsimd.indirect_dma_start(
                        out=g[:],
                        out_offset=None,
                        in_=qf[p][:],
                        in_offset=bass.IndirectOffsetOnAxis(
                            ap=idx_s[:, col : col + 1], axis=0
                        ),
                    )
                    nc.tensor.matmul(
                        out=pp3[:],
                        lhsT=g[:],
                        rhs=s_s[:, col * P : (col + 1) * P],
                        start=(c == 0),
                        stop=(c == int(n3[t][p]) - 1),
                    )
                dstv = acc[:, t * P : (t + 1) * P]
                if not acc_started[t]:
                    nc.vector.tensor_copy(out=dstv, in_=pp3[:])
                    acc_started[t] = True
                else:
                    nc.vector.tensor_tensor(
                        out=dstv, in0=dstv, in1=pp3[:],
                        op=mybir.AluOpType.add,
                    )

            # ---------------- Phase 2: layer 2 + Q (+ interleaved phase 3) --
            # Software-pipelined: iteration t aggregates tile t and GEMMs
            # tile t-1, so the PSUM->SBUF copy hides under the previous GEMM.
            p3_queue = []  # (ready_iter, t3, p)
            aggT2s = [None, None]
            for t in range(NT2 + 1):
                if t < NT2:
                    idx_s = mp.tile([P, ECH2], i32, tag="idx")
                    nc.sync.dma_start(out=idx_s[:], in_=idx2_t[t])
                    s_s = s2p.tile([P, ECH2 * P], bf16, tag="s2")
                    nc.sync.dma_start(out=s_s[:], in_=s2_t[t])

                    pa2 = pa2p.tile([P, H], f32, tag="pa2", name="pa2")
                    gs = []
                    for c in range(ECH2):
                        g = gp.tile([P, H], bf16, tag="g")
                        nc.gpsimd.indirect_dma_start(
                            out=g[:],
                            out_offset=None,
                            in_=h1loc[:],
                            in_offset=bass.IndirectOffsetOnAxis(
                                ap=idx_s[:, c : c + 1], axis=0
                            ),
                        )
                        gs.append(g)
                    for j in range(NFI2):
                        for c in range(ECH2):
                            nc.tensor.matmul(
                                out=pa2[:, j * P : (j + 1) * P],
                                lhsT=gs[c][:, j * P : (j + 1) * P],
                                rhs=s_s[:, c * P : (c + 1) * P],
                                start=(c == 0),
                                stop=(c == ECH2 - 1),
                            )
                    aggT2 = ap.tile([P, H], bf16, tag="agg", name="aggT2")
                    nc.vector.tensor_copy(out=aggT2[:], in_=pa2[:])
                    aggT2s[t % 2] = aggT2

                if t >= 1:
                    tg = t - 1
                    aggT2g = aggT2s[tg % 2]
                    h2cs = []
                    for fo in range(NFI2):
                        pz = pz2p.tile([P, P], f32, tag="z", name="pz2")
                        for fi in range(NFI2):
                            nc.tensor.matmul(
                                out=pz[:],
                                lhsT=w2sb[fi][:, fo * P : (fo + 1) * P],
                                rhs=aggT2g[:, fi * P : (fi + 1) * P],
                                start=(fi == 0),
                                stop=(fi == NFI2 - 1),
                            )
                        h2c = h2p.tile([P, P], bf16, tag="h2c")
                        if use_b2:
                            nc.scalar.activation(
                                out=h2c[:], in_=pz[:], func=relu,
                                bias=b2sb[:, fo : fo + 1],
                            )
                        else:
                            nc.scalar.activation(out=h2c[:], in_=pz[:], func=relu)
                        h2cs.append(h2c)
                    pq = pqdp.tile([P, B], f32, tag="b", name="pq")
                    for fo in range(NFI2):
                        nc.tensor.matmul(
                            out=pq[:],
                            lhsT=h2cs[fo][:],
                            rhs=w3sb[fo][:],
                            start=(fo == 0),
                            stop=(fo == NFI2 - 1),
                        )
                    qn = mp.tile([P, B], bf16, tag="qn")
                    nc.vector.tensor_copy(out=qn[:], in_=pq[:])
                    nc.sync.dma_start(
                        out=q_slab[tg * P : (tg + 1) * P, :], in_=qn[:]
                    )
                    if debug:
                        nc.sync.dma_start(
                            out=q_dbg[tg * P : (tg + 1) * P, :], in_=qn[:]
                        )

                    if (tg + 1) in TBE:
                        ch = int(np.searchsorted(TBE, tg + 1))
                        nc.gpsimd.collective_compute(
                            "AllGather",
                            mybir.AluOpType.bypass,
                            replica_groups=rg,
                            ins=[q_slab[TBS[ch] * P : TBE[ch] * P, :]],
                            outs=[qf[ch][:]],
                        )
                        phase3_load(ch)
                        if ch < QCH - 1:
                            # drain only 2+ iterations later so the AllGather
                            # is done before Pool reaches these gathers
                            # (in-order SEQ would head-of-line block phase 2)
                            p3_queue.extend(
                                (t + 2, t3, ch) for t3 in range(NT2)
                            )

                drained = 0
                dmax = 11 if t >= 16 else 7
                while p3_queue and p3_queue[0][0] <= t and drained < dmax:
                    _, t3, pch = p3_queue.pop(0)
                    phase3_pass(t3, pch)
                    drained += 1

            for _, t3, pch in p3_queue:
                phase3_pass(t3, pch)
            for t3 in range(NT2):
                phase3_pass(t3, QCH - 1)

            nc.sync.dma_start(out=out_t[:], in_=acc[:])

    nc.finalize()
    return nc


_CACHE: dict = {}


def kernel(**inputs: np.ndarray) -> np.ndarray:
    nodes = np.asarray(inputs["nodes"], dtype=np.float32)
    edge_index = np.asarray(inputs["edge_index"])
    img = np.asarray(inputs["img"], dtype=np.float32)
    W1 = np.asarray(inputs["W1"], dtype=np.float32)
    b1 = np.asarray(inputs["b1"], dtype=np.float32)
    W2 = np.asarray(inputs["W2"], dtype=np.float32)
    b2 = np.asarray(inputs["b2"], dtype=np.float32)
    W3 = np.asarray(inputs["W3"], dtype=np.float32)
    b3 = np.asarray(inputs["b3"], dtype=np.float32)

    pp = _preprocess(edge_index)
    T1, ECH1, ECH2, NCH3 = pp["T1"], pp["ECH1"], pp["ECH2"], pp["NCH3"]
    use_b1 = bool(np.any(b1))
    use_b2 = bool(np.any(b2))

    key = (T1, ECH1, ECH2, NCH3, pp["n3"].tobytes(), use_b1, use_b2)
    if key not in _CACHE:
        _CACHE[key] = _build(T1, ECH1, ECH2, NCH3, pp["n3"], use_b1, use_b2)
    nc = _CACHE[key]

    nodes_bf = nodes.astype(bf)
    w1_bf = W1.astype(bf)
    w2_bf = W2.astype(bf)
    w3img = (W3 @ img.T).astype(bf)  # [H, B]
    outbias = img @ b3  # [B]

    in_maps = []
    for k in range(NCORES):
        et1, srcg, cidx1, lane1 = pp["l1_gather"][k]
        xg = np.zeros((T1, P, ECH1, D), bf)
        xg[et1, lane1, cidx1] = nodes_bf[srcg]
        m = {
            "xg": xg.reshape(T1, P, ECH1 * D),
            "S1": pp["S1_l"][k].astype(bf),
            "S2": np.ascontiguousarray(pp["S2"][k]).astype(bf),
            "S3": np.ascontiguousarray(pp["S3"][k]).astype(bf),
            "idx2": np.ascontiguousarray(pp["idx2"][k]),
            "idx3": np.ascontiguousarray(pp["idx3"][k]),
            "W1": w1_bf,
            "W2": w2_bf,
            "W3img": w3img,
        }
        if use_b1:
            m["b1"] = b1.reshape(1, H).astype(bf)
        if use_b2:
            m["b2"] = np.ascontiguousarray(b2.reshape(NFI2, P).T).astype(np.float32)
        in_maps.append(m)

    res = run_bass_kernel_spmd(nc, in_maps, core_ids=list(range(NCORES)))

    full = np.concatenate([res.results[k]["out"] for k in range(NCORES)], axis=1)
    cols = pp["colmap"][np.arange(N_SKIP, N)]
    out = full[:, cols] + outbias[:, None]
    return out.astype(np.float32)


if __name__ == "__main__":
    rng = np.random.default_rng(0)
    ins = {
        "nodes": rng.standard_normal((N, D)).astype(np.float32),
        "edge_index": rng.integers(0, N, size=(2, E)).astype(np.int64),
        "img": rng.standard_normal((B, D)).astype(np.float32),
        "W1": (rng.standard_normal((D, H)) * 0.02).astype(np.float32),
        "b1": np.zeros(H, np.float32),
        "W2": (rng.standard_normal((H, H)) * 0.02).astype(np.float32),
        "b2": np.zeros(H, np.float32),
        "W3": (rng.standard_normal((H, D)) * 0.02).astype(np.float32),
        "b3": np.zeros(D, np.float32),
    }
    out = kernel(**ins)
    print("out", out.shape, out.dtype, np.abs(out).mean())
